# revision 26
# baseline (speedup 1.0000x reference)
"""BiMamba block Trainium2 kernel.

Sharding: 8 cores = (batch 2) x (direction 2) x (d_inner half 2).
Each core runs an identical SPMD program; per-core weights/data encode
(b, dir, h). Host does the final linear gather-sum (+ proj bias + residual).

Per-core layout is channel-major: activations live as (feature, time) tiles so
all matmuls chain without transposes, the causal conv is free-dim shifts, and
the SSM scan runs along the free dim via tensor_tensor_scan.

The SSM inner loop runs entirely on DVE (big GPSIMD streams and DVE throttle
each other ~3-4x on SBUF, so GPSIMD only gets the tiny y3 gate + hend copy):
dA via a squaring ladder from rho = exp(-delta) (A_n = -(n+1) exactly; rho is
computed on ScalarE once per chunk next to the softplus so the exp/ln table
never swaps against the silu table mid-loop), then dBu, scan, C-mult and a
4-level n-reduction tree. Phase work for chunk c+1 (LN on ScalarE accum_out,
in_proj, conv, x_proj, dt+rho+B/C replication) is emitted in slices inside
chunk c's m-loop so PE/ACT/DMA overlap the DVE-bound scan loop, and the fused
out_proj+final_proj accumulates k-major into a PSUM tile held per chunk.
"""

import numpy as np
import ml_dtypes

import concourse.bass as bass
import concourse.bacc as bacc
import concourse.tile as tile
from concourse import mybir
from concourse.bass_utils import run_bass_kernel_spmd
from concourse.masks import make_identity
from concourse import hw_specs as _hw_specs

# Pin activation tables: keep only the exp+ln set and the silu set populated
# (order preserved so act_func_set ids stay consistent with walrus); all other
# sets become empty so the table-load inserter can never alternate between
# e.g. exp_and_others and natural_log for a softplus Exp/Ln pair.
_KEEP_TABLES = {"natural_log_exp_and_others", "silu_and_others"}
_orig_gat = _hw_specs.get_activation_tables


def _gat(arch):
    d = _orig_gat(arch)
    return {k: (v if k in _KEEP_TABLES else set()) for k, v in d.items()}


_hw_specs.get_activation_tables = _gat
bacc.get_activation_tables = _gat

F32 = mybir.dt.float32
BF16 = mybir.dt.bfloat16
AL = mybir.AluOpType
AF = mybir.ActivationFunctionType

D_MODEL = 768
D_STATE = 16
D_CONV = 4
D_INNER = 1536
DT_RANK = 48
B_SZ = 2
SEQ = 1024

HALF = 768           # d_inner half per core
TC = 256             # time chunk
NCH = SEQ // TC
NT = TC // 128       # 128-row time tiles per chunk

# packed per-partition f32 column offsets in `cols` (128, NCOL)
G0 = 0               # norm_g (6)
BL0 = 6              # norm_b (6)
CW0 = 12             # conv_w (12 blocks x 4 taps)
CB0 = 60             # conv_b (12)
DTB0 = 72            # dt bias (6)
DP0 = 78             # D param (6)
AA0 = 84             # A = -exp(A_log) (6 blocks x 16)
EPS0 = 178           # layernorm eps
NCOL = 180

_compiled = {}


def _col(cols, i):
    return cols[:, i:i + 1]


def _bcast_n(ap2d, n):
    """(128, T) AP -> (128, n, T) view with stride-0 n dim."""
    return bass.AP(tensor=ap2d.tensor, offset=ap2d.offset,
                   ap=[ap2d.ap[0], [0, n], ap2d.ap[1]])


def _flat(ap3d):
    return ap3d.rearrange("p n t -> p (n t)")


def _build_nc(a_chain):
    nc = bacc.Bacc("TRN2", target_bir_lowering=False, num_devices=8)

    xin_d = nc.dram_tensor("xin", [SEQ, D_MODEL], F32, kind="ExternalInput")
    w_iz_d = nc.dram_tensor("w_iz", [128, 6, 2304], BF16, kind="ExternalInput")
    w_xp_d = nc.dram_tensor("w_xp", [128, 12, 112], BF16, kind="ExternalInput")
    w_dt_d = nc.dram_tensor("w_dt", [48, 768], BF16, kind="ExternalInput")
    w_f_d = nc.dram_tensor("w_f", [128, 6, 768], BF16, kind="ExternalInput")
    cols_d = nc.dram_tensor("cols", [128, NCOL], F32, kind="ExternalInput")
    diag_d = nc.dram_tensor("diag", [128, 48, 128], BF16, kind="ExternalInput")
    out_d = nc.dram_tensor("out", [128, 6, SEQ], F32, kind="ExternalOutput")

    from contextlib import ExitStack
    with ExitStack() as ctx:
        tc = ctx.enter_context(tile.TileContext(nc))
        wp = ctx.enter_context(tc.tile_pool(name="wp", bufs=1))
        lnp = ctx.enter_context(tc.tile_pool(name="lnp", bufs=2))
        lsp = ctx.enter_context(tc.tile_pool(name="lsp", bufs=2))
        xnp = ctx.enter_context(tc.tile_pool(name="xnp", bufs=2))
        xip = ctx.enter_context(tc.tile_pool(name="xip", bufs=2))
        szp = ctx.enter_context(tc.tile_pool(name="szp", bufs=2))
        xsp = ctx.enter_context(tc.tile_pool(name="xsp", bufs=2))
        bcp = ctx.enter_context(tc.tile_pool(name="bcp", bufs=2))
        dlp = ctx.enter_context(tc.tile_pool(name="dlp", bufs=2))
        rop = ctx.enter_context(tc.tile_pool(name="rop", bufs=2))
        repp = ctx.enter_context(tc.tile_pool(name="rep", bufs=1))
        adp = ctx.enter_context(tc.tile_pool(name="adp", bufs=2))
        bup = ctx.enter_context(tc.tile_pool(name="bup", bufs=1))
        hp = ctx.enter_context(tc.tile_pool(name="hp", bufs=2))
        gp = ctx.enter_context(tc.tile_pool(name="gp", bufs=2))
        tp = ctx.enter_context(tc.tile_pool(name="tp", bufs=1))
        wcp = ctx.enter_context(tc.tile_pool(name="wcp", bufs=2))
        y3p = ctx.enter_context(tc.tile_pool(name="y3p", bufs=2))
        outp = ctx.enter_context(tc.tile_pool(name="outp", bufs=2))
        drp = ctx.enter_context(tc.tile_pool(name="drp", bufs=2, space="DRAM"))
        pmP = ctx.enter_context(tc.tile_pool(name="pmP", bufs=2, space="PSUM"))
        ptrP = ctx.enter_context(tc.tile_pool(name="ptrP", bufs=1, space="PSUM"))
        miscP = ctx.enter_context(tc.tile_pool(name="miscP", bufs=1, space="PSUM"))
        poP = ctx.enter_context(tc.tile_pool(name="poP", bufs=1, space="PSUM"))

        w_iz = wp.tile([128, 6, 2304], BF16, tag="w_iz")
        w_xp = wp.tile([128, 12, 112], BF16, tag="w_xp")
        w_dt = wp.tile([48, 768], BF16, tag="w_dt")
        w_f = wp.tile([128, 6, 768], BF16, tag="w_f")
        cols = wp.tile([128, NCOL], F32, tag="cols")
        diag = wp.tile([128, 48, 128], BF16, tag="diag")
        ident = wp.tile([128, 128], F32, tag="ident")
        hend = wp.tile([128, 6, 16], F32, tag="hend")
        # cols first (LN needs it), then in_proj weights; the rest after
        nc.sync.dma_start(out=cols[:], in_=cols_d[:])
        nc.sync.dma_start(out=w_iz[:], in_=w_iz_d[:])
        nc.sync.dma_start(out=diag[:], in_=diag_d[:])
        nc.sync.dma_start(out=w_xp[:], in_=w_xp_d[:])
        nc.sync.dma_start(out=w_dt[:], in_=w_dt_d[:])
        nc.sync.dma_start(out=w_f[:], in_=w_f_d[:])
        make_identity(nc, ident[:])

        S = {}               # per-chunk tile handles

        def emit_ln(c, tt):
            t0 = c * TC
            if tt == 0:
                xnc = xnp.tile([128, 6, TC], BF16, tag="xnc")
                S[c] = {"xnc": xnc}
            xnc = S[c]["xnc"]
            xt = lnp.tile([128, D_MODEL], F32, tag="xt")
            # xin loads ride the ACT hwdge queue so they never queue behind
            # the big weight DMAs on the sync queue
            nc.scalar.dma_start(out=xt[:], in_=xin_d[t0 + tt * 128:t0 + (tt + 1) * 128, :])
            xtn = lnp.tile([128, D_MODEL], F32, tag="xtn")
            if c == 0:
                # prologue: DVE is idle, use bn_stats and keep ACT short
                st = lsp.tile([128, 3, 6], F32, tag="st")
                for sg in range(3):
                    nc.vector.bn_stats(out=st[:, sg, :],
                                       in_=xt[:, sg * 256:(sg + 1) * 256])
                mv = lsp.tile([128, 2], F32, tag="mv")
                nc.vector.bn_aggr(out=mv[:], in_=st[:])
                rs = lsp.tile([128, 1], F32, tag="rs")
                nc.scalar.activation(rs[:], mv[:, 1:2], AF.Ln, bias=_col(cols, EPS0))
                nc.scalar.activation(rs[:], rs[:], AF.Exp, scale=-0.5)
                nc.vector.tensor_scalar(xtn[:], xt[:], mv[:, 0:1], rs[:],
                                        AL.subtract, AL.mult)
            else:
                # LN stats on ACT (accum_out) keep DVE free for the scan loop
                s1 = lsp.tile([128, 1], F32, tag="s1")
                s2 = lsp.tile([128, 1], F32, tag="s2")
                nc.scalar.activation(xtn[:], xt[:], AF.Identity, accum_out=s1[:])
                nc.scalar.activation(xtn[:], xt[:], AF.Square, accum_out=s2[:])
                q = lsp.tile([128, 1], F32, tag="q")
                nc.vector.tensor_tensor(q[:], s1[:], s1[:], AL.mult)
                bm = lsp.tile([128, 1], F32, tag="bm")
                # var + eps = s2/768 - (s1/768)^2 + eps
                nc.vector.tensor_scalar(bm[:], q[:], -1.0 / (D_MODEL * D_MODEL),
                                        1e-5, AL.mult, AL.add)
                rs = lsp.tile([128, 1], F32, tag="rs")
                nc.scalar.activation(rs[:], s2[:], AF.Ln, bias=bm[:],
                                     scale=1.0 / D_MODEL)
                nc.scalar.activation(rs[:], rs[:], AF.Exp, scale=-0.5)
                v = lsp.tile([128, 1], F32, tag="v")
                nc.vector.tensor_scalar(v[:], s1[:], -1.0 / D_MODEL, 0.0,
                                        AL.mult, AL.add)
                nmu = lsp.tile([128, 1], F32, tag="nmu")
                nc.vector.tensor_tensor(nmu[:], v[:], rs[:], AL.mult)
                nc.scalar.activation(xtn[:], xt[:], AF.Identity, bias=nmu[:],
                                     scale=rs[:])
            for dk in range(6):
                pt = ptrP.tile([128, 128], F32, tag="pt")
                nc.tensor.transpose(pt[:], xtn[:, dk * 128:(dk + 1) * 128], ident[:])
                nc.scalar.activation(xnc[:, dk, tt * 128:(tt + 1) * 128], pt[:],
                                     AF.Identity, bias=_col(cols, BL0 + dk),
                                     scale=_col(cols, G0 + dk))

        def emit_inproj(c, lo, hi):
            xnc = S[c]["xnc"]
            if lo == 0:
                xic = xip.tile([128, 12, TC + 3], BF16, tag="xic")
                S[c]["xic"] = xic
                szc = szp.tile([128, 6, TC], BF16, tag="szc")
                S[c]["szc"] = szc
                if c == 0:
                    nc.vector.memset(xic[:, :, 0:3], 0.0)
                else:
                    nc.vector.tensor_copy(xic[:, :, 0:3],
                                          S[c - 1]["xic"][:, :, TC:TC + 3])
            xic, szc = S[c]["xic"], S[c]["szc"]
            for m in range(lo, hi):
                pm = pmP.tile([128, TC], F32, tag="pm")
                for k in range(6):
                    nc.tensor.matmul(pm[:], w_iz[:, k, m * 128:(m + 1) * 128],
                                     xnc[:, k, :], start=(k == 0), stop=(k == 5))
                if m < 12:
                    if c == 0:
                        # prologue: DVE is idle, keep the ACT queue short
                        nc.vector.tensor_copy(xic[:, m, 3:3 + TC], pm[:])
                    else:
                        nc.scalar.copy(xic[:, m, 3:3 + TC], pm[:])
                else:
                    nc.scalar.activation(szc[:, m - 12, :], pm[:], AF.Silu)

        def emit_conv(c):
            xic = S[c]["xic"]
            xsc = xsp.tile([128, 12, TC], BF16, tag="xsc")
            S[c]["xsc"] = xsc
            for m in range(12):
                pc = pmP.tile([128, TC], F32, tag="pm")
                for k in range(4):
                    nc.tensor.matmul(pc[:], diag[:, m * 4 + k, :], xic[:, m, k:k + TC],
                                     start=(k == 0), stop=(k == 3))
                nc.scalar.activation(xsc[:, m, :], pc[:], AF.Silu,
                                     bias=_col(cols, CB0 + m))

        def emit_xproj(c):
            xsc = S[c]["xsc"]
            pxp = miscP.tile([112, TC], F32, tag="pxp")
            for k in range(12):
                nc.tensor.matmul(pxp[:], w_xp[:, k, :], xsc[:, k, :],
                                 start=(k == 0), stop=(k == 11))
            bro = bcp.tile([16, TC], BF16, tag="bro")
            cro = bcp.tile([16, TC], BF16, tag="cro")
            dts = bcp.tile([48, TC], BF16, tag="dts")
            if c == 0:
                nc.vector.tensor_copy(bro[:], pxp[0:16, :])
                nc.vector.tensor_copy(cro[:], pxp[32:48, :])
                nc.vector.tensor_copy(dts[:], pxp[64:112, :])
            else:
                nc.scalar.copy(bro[:], pxp[0:16, :])
                nc.scalar.copy(cro[:], pxp[32:48, :])
                nc.scalar.copy(dts[:], pxp[64:112, :])
            S[c]["bro"], S[c]["cro"], S[c]["dts"] = bro, cro, dts

        def emit_dt_reps(c):
            dts = S[c]["dts"]
            dlc = dlp.tile([128, 6, TC], BF16, tag="dlc")
            S[c]["dlc"] = dlc
            for m in range(6):
                pd = miscP.tile([128, TC], F32, tag="pd")
                nc.tensor.matmul(pd[:], w_dt[:, m * 128:(m + 1) * 128], dts[:],
                                 start=True, stop=True)
                ee = dlp.tile([128, TC], F32, tag="ee")
                nc.scalar.activation(ee[:], pd[:], AF.Exp, bias=_col(cols, DTB0 + m))
                nc.scalar.activation(dlc[:, m, :], ee[:], AF.Ln, bias=1.0)
            if a_chain:
                # rho = exp(-delta) for all 6 blocks, grouped here so the SSM
                # loop emits no ACT work (exp table stays loaded; dA rows come
                # from a DVE squaring ladder)
                rho = rop.tile([128, 6, TC], BF16, tag="rho")
                S[c]["rho"] = rho
                for m in range(6):
                    nc.scalar.activation(rho[:, m, :], dlc[:, m, :], AF.Exp,
                                         scale=-1.0)
            # replicate B/C rows across partitions via DRAM round-trip
            brep = repp.tile([128, 16, TC], BF16, tag="brep")
            crep = repp.tile([128, 16, TC], BF16, tag="crep")
            S[c]["brep"], S[c]["crep"] = brep, crep
            for src, dst in ((S[c]["bro"], brep), (S[c]["cro"], crep)):
                scr = drp.tile([16, TC], BF16, tag="scr")
                nc.sync.dma_start(out=scr[:], in_=src[:])
                sv = scr[:]
                rd = bass.AP(tensor=sv.tensor, offset=sv.offset,
                             ap=[[0, 128], sv.ap[0], sv.ap[1]])
                nc.sync.dma_start(out=dst[:], in_=rd)

        pend = {}

        def emit_dA(c, m):
            # allocate dA(c, m) and emit its ScalarE exp rows ahead of any
            # phase-slice silus so the exps never queue behind them
            dlc = S[c]["dlc"]
            dA = adp.tile([128, 16, TC], BF16, tag="dA")
            pend[(c, m)] = dA
            lo = 8 if a_chain else 0
            for n in range(lo, 16):
                nc.scalar.activation(dA[:, n, :], dlc[:, m, :], AF.Exp,
                                     scale=_col(cols, AA0 + m * 16 + n))

        def emit_ssm(c, m):
            xsc, dlc = S[c]["xsc"], S[c]["dlc"]
            u = xsc[:, m, :]
            dA = pend.pop((c, m))
            if a_chain:
                # rows 0..7 by DVE squaring ladder from rho
                rho_m = S[c]["rho"][:, m, :]
                nc.vector.tensor_copy(dA[:, 0, :], rho_m)
                nc.vector.tensor_tensor(dA[:, 1, :], rho_m, rho_m, AL.mult)
                nc.vector.tensor_tensor(_flat(dA[:, 2:4, :]), _flat(dA[:, 0:2, :]),
                                        _bcast_n(dA[:, 1, :], 2), AL.mult)
                nc.vector.tensor_tensor(_flat(dA[:, 4:8, :]), _flat(dA[:, 0:4, :]),
                                        _bcast_n(dA[:, 3, :], 4), AL.mult)
            else:
                for n in range(16):
                    nc.scalar.activation(dA[:, n, :], dlc[:, m, :], AF.Exp,
                                         scale=_col(cols, AA0 + m * 16 + n))
            wc = wcp.tile([128, TC], BF16, tag="wc")
            nc.vector.tensor_tensor(wc[:], dlc[:, m, :], u, AL.mult)
            dBu = bup.tile([128, 16, TC], BF16, tag="dBu")
            nc.vector.tensor_tensor(_flat(dBu[:]), _bcast_n(wc[:], 16),
                                    _flat(S[c]["brep"][:]), AL.mult)
            if c > 0:
                fx = wcp.tile([128, 16], F32, tag="fx")
                nc.vector.tensor_tensor(fx[:], dA[:, :, 0], hend[:, m, :], AL.mult)
                nc.vector.tensor_tensor(dBu[:, :, 0], dBu[:, :, 0], fx[:], AL.add)
            nc.vector.memset(dA[:, :, 0], 0.0)
            h = hp.tile([128, 16, TC], BF16, tag="h")
            nc.vector.tensor_tensor_scan(_flat(h[:]), _flat(dA[:]),
                                         _flat(dBu[:]), 0.0, AL.mult, AL.add)
            nc.gpsimd.tensor_copy(hend[:, m, :], h[:, :, TC - 1])
            # g = C * h and the n-reduction, all on DVE (GPSIMD and DVE
            # throttle each other heavily when streaming concurrently);
            # distinct dst rows at each tree level keep 2x packing
            g = gp.tile([128, 16, TC], BF16, tag="g")
            nc.vector.tensor_tensor(_flat(g[:]), _flat(h[:]),
                                    _flat(S[c]["crep"][:]), AL.mult)
            ts = tp.tile([128, 8, TC], BF16, tag="ts")
            nc.vector.tensor_tensor(_flat(ts[:, 0:8, :]), _flat(g[:, 0:8, :]),
                                    _flat(g[:, 8:16, :]), AL.add)
            nc.vector.tensor_tensor(_flat(g[:, 0:4, :]), _flat(ts[:, 0:4, :]),
                                    _flat(ts[:, 4:8, :]), AL.add)
            nc.vector.tensor_tensor(_flat(g[:, 4:6, :]), _flat(g[:, 0:2, :]),
                                    _flat(g[:, 2:4, :]), AL.add)
            nc.vector.tensor_tensor(g[:, 6, :], g[:, 4, :], g[:, 5, :], AL.add)
            # y2 = u*D + y ; y3 = y2 * silu(z) (tiny, on GPSIMD)
            nc.vector.scalar_tensor_tensor(g[:, 7, :], u, _col(cols, DP0 + m),
                                           g[:, 6, :], AL.mult, AL.add)
            nc.gpsimd.tensor_tensor(S[c]["y3c"][:, m, :], g[:, 7, :],
                                    S[c]["szc"][:, m, :], AL.mult)
            # k-major slice of the fused out_proj+final_proj: accumulate this
            # y3 row into all 6 output blocks (PSUM held across the chunk)
            po = S[c]["po"]
            for mm in range(6):
                nc.tensor.matmul(po[:, mm, :], w_f[:, m, mm * 128:(mm + 1) * 128],
                                 S[c]["y3c"][:, m, :], start=(m == 0), stop=(m == 5))

        def emit_wf(c):
            t0 = c * TC
            po = S[c]["po"]
            for m in range(6):
                ob = outp.tile([128, TC], F32, tag="ob")
                nc.scalar.copy(ob[:], po[:, m, :])
                nc.sync.dma_start(out=out_d[:, m, t0:t0 + TC], in_=ob[:])

        # prologue: full phase for chunk 0
        emit_ln(0, 0)
        emit_ln(0, 1)
        emit_inproj(0, 0, 12)
        emit_conv(0)
        emit_xproj(0)
        emit_dt_reps(0)
        emit_inproj(0, 12, 18)
        for c in range(NCH):
            y3c = y3p.tile([128, 6, TC], BF16, tag="y3c")
            S[c]["y3c"] = y3c
            po = poP.tile([128, 6, TC], F32, tag="po")
            S[c]["po"] = po
            if c > 0:
                emit_wf(c - 1)     # w_f copies of previous chunk
            if c == 0:
                emit_dA(0, 0)
            for m in range(6):
                emit_ssm(c, m)
                if m + 1 < 6:
                    emit_dA(c, m + 1)
                if c + 1 < NCH:
                    if m == 0:
                        emit_ln(c + 1, 0)
                    elif m == 1:
                        emit_ln(c + 1, 1)
                    elif m == 2:
                        emit_inproj(c + 1, 0, 9)
                    elif m == 3:
                        emit_inproj(c + 1, 9, 18)
                        emit_conv(c + 1)
                        emit_xproj(c + 1)
                    elif m == 4:
                        emit_dt_reps(c + 1)
                    elif m == 5:
                        emit_dA(c + 1, 0)
            if c >= 2:
                del S[c - 2]
        emit_wf(NCH - 1)

    nc.finalize()
    return nc


def _to_sb(w, nblk):
    """(nblk*128, X) -> (128, nblk, X) partition-major layout."""
    x = w.shape[1]
    return np.ascontiguousarray(w.reshape(nblk, 128, x).transpose(1, 0, 2))


def _cols_vec(v, nblk):
    """(nblk*128,) -> (128, nblk)."""
    return np.ascontiguousarray(v.reshape(nblk, 128).T)


def _bf(a):
    return np.ascontiguousarray(a.astype(ml_dtypes.bfloat16))


def _prep_weight_set(p, proj_w, h):
    """p: dict of one direction's mamba params; returns per-core DRAM arrays."""
    sl = slice(h * HALF, (h + 1) * HALF)
    # permute d_inner so own half comes first
    perm = np.concatenate([np.arange(h * HALF, (h + 1) * HALF),
                           np.arange((1 - h) * HALF, (2 - h) * HALF)])
    in_w = np.asarray(p["in_w"], np.float32)
    xi_w = in_w[:D_INNER][perm]                       # (1536, 768)
    z_w = in_w[D_INNER:][sl]                          # (768, 768)
    w_iz = np.concatenate([xi_w, z_w], axis=0).T      # (768, 2304)
    w_iz = _bf(_to_sb(w_iz, 6))

    xp = np.asarray(p["xproj_w"], np.float32)
    xp_pad = np.zeros((112, D_INNER), np.float32)
    xp_pad[0:16] = xp[DT_RANK:DT_RANK + 16]           # B
    xp_pad[32:48] = xp[DT_RANK + 16:DT_RANK + 32]     # C
    xp_pad[64:112] = xp[0:DT_RANK]                    # dt
    w_xp = _bf(_to_sb(np.ascontiguousarray(xp_pad[:, perm].T), 12))  # (128,12,112)

    dt_w = np.asarray(p["dt_w"], np.float32)[sl]      # (768, 48)
    w_dt = _bf(np.ascontiguousarray(dt_w.T))          # (48, 768)

    out_w = np.asarray(p["out_w"], np.float32)        # (768, 1536)
    w_fold = proj_w @ out_w[:, sl]                    # (768dm, 768dy)
    w_f = _bf(_to_sb(np.ascontiguousarray(w_fold.T), 6))  # (128, 6, 768)

    conv_w = np.asarray(p["conv_w"], np.float32)[perm]    # (1536, 4)
    conv_b = np.asarray(p["conv_b"], np.float32)[perm]
    dt_b = np.asarray(p["dt_b"], np.float32)[sl]
    A = -np.exp(np.asarray(p["A_log"], np.float32))[sl]   # (768, 16)
    Dp = np.asarray(p["D"], np.float32)[sl]
    return w_iz, w_xp, w_dt, w_f, conv_w, conv_b, dt_b, A, Dp


def kernel(**inputs):
    a_all = np.stack([-np.exp(np.asarray(inputs[p + "A_log"], np.float32)) for p in ("f_", "b_")])
    a_chain = bool(np.allclose(a_all, -np.arange(1, 17, dtype=np.float32)[None, None, :],
                               rtol=1e-6, atol=1e-6))
    key = ("nc", a_chain)
    if key not in _compiled:
        _compiled[key] = _build_nc(a_chain)
    nc = _compiled[key]

    x = np.asarray(inputs["x"], np.float32)
    norm_g = np.asarray(inputs["norm_g"], np.float32)
    norm_b = np.asarray(inputs["norm_b"], np.float32)
    proj_w = np.asarray(inputs["proj_w"], np.float32)
    proj_b = np.asarray(inputs["proj_b"], np.float32)

    # 4 distinct weight sets: (dir, h); shared across batch
    wsets = {}
    for d in range(2):
        pref = "f_" if d == 0 else "b_"
        p = {k: inputs[pref + k] for k in
             ("in_w", "conv_w", "conv_b", "xproj_w", "dt_w", "dt_b", "A_log", "D", "out_w")}
        pw_half = proj_w[:, d * D_MODEL:(d + 1) * D_MODEL]
        for h in range(2):
            w_iz, w_xp, w_dt, w_f, conv_w, conv_b, dt_b, A, Dp = \
                _prep_weight_set(p, pw_half, h)
            cols = np.zeros((128, NCOL), np.float32)
            cols[:, G0:G0 + 6] = _cols_vec(norm_g, 6)
            cols[:, BL0:BL0 + 6] = _cols_vec(norm_b, 6)
            cols[:, CW0:CW0 + 48] = conv_w.reshape(12, 128, 4).transpose(1, 0, 2).reshape(128, 48)
            cols[:, CB0:CB0 + 12] = _cols_vec(conv_b, 12)
            cols[:, DTB0:DTB0 + 6] = _cols_vec(dt_b, 6)
            cols[:, DP0:DP0 + 6] = _cols_vec(Dp, 6)
            cols[:, AA0:AA0 + 96] = A.reshape(6, 128, 16).transpose(1, 0, 2).reshape(128, 96)
            cols[:, EPS0] = 1e-5
            diag = np.zeros((128, 48, 128), ml_dtypes.bfloat16)
            cwp = conv_w.reshape(12, 128, 4)
            for m in range(12):
                for k in range(4):
                    np.fill_diagonal(diag[:, m * 4 + k, :], cwp[m, :, k])
            wsets[(d, h)] = dict(w_iz=w_iz, w_xp=w_xp, w_dt=w_dt, w_f=w_f,
                                 cols=np.ascontiguousarray(cols),
                                 diag=np.ascontiguousarray(diag))

    in_maps = []
    meta = []
    for b in range(2):
        for d in range(2):
            xb = x[b] if d == 0 else x[b, ::-1]
            xb = np.ascontiguousarray(xb)
            for h in range(2):
                im = dict(wsets[(d, h)])
                im["xin"] = xb
                in_maps.append(im)
                meta.append((b, d, h))

    _compiled["last_in_maps"] = in_maps
    res = run_bass_kernel_spmd(nc, in_maps, core_ids=list(range(8)))

    out = np.tile((proj_b[None, :]).astype(np.float32), (B_SZ, SEQ, 1)) + x
    for i, (b, d, h) in enumerate(meta):
        po = res.results[i]["out"]                     # (128, 6, 1024)
        po = po.transpose(2, 1, 0).reshape(SEQ, D_MODEL)   # (t, dm)
        if d == 1:
            po = po[::-1]
        out[b] += po
    return out
